# revision 1
# baseline (speedup 1.0000x reference)
"""ChaosNet (ChaosFEX + linear head) Trainium2 kernel.

Math restructure: every per-element feature depends only on k*(x) = first
trajectory index k with |traj[k] - x| < eps.  k*(x) is piecewise-constant in x
(first-claim intervals of the shared trajectory), so the model output

    out[n, c] = b_c + sum_f Phi_{c,f}(k*(x[n,f]))

is, per (c, f), a piecewise-constant function of x with M segments.  With
region left-edges L_0 <= ... <= L_{M-1} and per-segment table values Phi[m],
a telescoped form needs only rank indicators:

    Phi(x) = sum_m [x >= L_m] * dPhi[m]          (dPhi = successive deltas)

On device (per core, 256 rows of x):
  - gpsimd broadcasts x (f-major [1, 8192]) across 128 partitions
  - vector engine computes u[m, j] = (x[j] >= L[m])  in fp32 {0,1}
  - tensor engine accumulates out[c, n] += sum_m u[m, f*256+n] * dPhi[m, 2f+c]
    over all 32 f-blocks into one [2, 256] PSUM tile
  - bias add, DMA out.

The host does only the inherently sequential scalar work: the 10000-step
trajectory, its prefix sums, and the exact-fp32 region partition (binary
search on fp32 bit patterns, so region edges reproduce the reference's
fp32 comparison semantics exactly).
"""

import os
import sys
from contextlib import ExitStack

import numpy as np

sys.path.insert(0, "/opt/trn_rl_repo")

import concourse.bass as bass  # noqa: E402
import concourse.tile as tile  # noqa: E402
from concourse import bacc, mybir  # noqa: E402
from concourse.bass_utils import run_bass_kernel_spmd  # noqa: E402

T = 10000
N = 2048
F = 32
NCORES = 8
N_LOC = N // NCORES            # 256 rows per core
E = N_LOC * F                  # 8192 elements per core
MCHUNK = 128                   # region-table rows per partition chunk
SLICE = 1024                   # free-dim pipeline slice (4 f-blocks)

np.seterr(all="ignore")

LAST_RESULTS = None            # BassKernelResults of the most recent run
LAST_NC = None                 # compiled Bass program of the most recent run


# ----------------------------------------------------------------------------
# Host-side preprocessing
# ----------------------------------------------------------------------------

def _build_traj(ic, thr):
    """fp32 skew-tent trajectory, bit-identical to the jax scan."""
    traj = np.empty(T, np.float32)
    z = np.float32(ic)
    thr = np.float32(thr)
    one = np.float32(1.0)
    omt = np.float32(one - thr)
    for k in range(T):
        traj[k] = z
        z = np.float32(z / thr) if z < thr else np.float32((one - z) / omt)
    return traj


def _sortable(i):
    """int32 bit pattern -> order-isomorphic int32 key (handles negatives)."""
    return np.where(i >= 0, i, i ^ np.int32(0x7FFFFFFF))


def _unsortable(k):
    return np.where(k >= 0, k, k ^ np.int32(0x7FFFFFFF))


def _match_intervals(traj, eps, xmin, xmax):
    """Exact fp32 interval [lo_k, hi_k] of {x in [xmin,xmax] :
    |fl32(traj_k - x)| < eps}; valid[k]=False if empty."""
    eps = np.float32(eps)
    xmin = np.float32(xmin)
    xmax = np.float32(xmax)

    def cond(xs):
        return np.abs(traj - xs.astype(np.float32)) < eps

    anchor = np.clip(traj, xmin, xmax)
    valid = cond(anchor)

    I = lambda f: _sortable(f.view(np.int32))             # noqa: E731
    Fv = lambda k: _unsortable(k).view(np.float32)        # noqa: E731

    def bisect(lo_i, hi_i, need, want_smallest_true):
        # invariant: cond(Fv(hi_i)) True/False per direction; int keys.
        for _ in range(40):
            gap = np.where(need, hi_i - lo_i, 0)
            if (gap <= 1).all():
                break
            mid = ((lo_i.astype(np.int64) + hi_i) // 2).astype(np.int32)
            cm = cond(Fv(mid))
            if want_smallest_true:
                hi_i = np.where(need & cm, mid, hi_i)
                lo_i = np.where(need & ~cm, mid, lo_i)
            else:
                lo_i = np.where(need & cm, mid, lo_i)
                hi_i = np.where(need & ~cm, mid, hi_i)
        return lo_i, hi_i

    # left edge: smallest x in [xmin, anchor] with cond True
    at_min = cond(np.full(T, xmin, np.float32))
    lo_edge = np.where(at_min, xmin, np.float32(np.nan))
    need = valid & np.isnan(lo_edge)
    lo_i = np.broadcast_to(I(xmin.reshape(1)), (T,)).copy()
    hi_i = I(anchor.copy())
    lo_i, hi_i = bisect(lo_i, hi_i, need, True)
    lo_edge = np.where(np.isnan(lo_edge), Fv(hi_i), lo_edge)

    # right edge: largest x in [anchor, xmax] with cond True
    at_max = cond(np.full(T, xmax, np.float32))
    hi_edge = np.where(at_max, xmax, np.float32(np.nan))
    need = valid & np.isnan(hi_edge)
    lo_i = I(anchor.copy())
    hi_i = np.broadcast_to(I(xmax.reshape(1)), (T,)).copy()
    lo_i, hi_i = bisect(lo_i, hi_i, need, False)
    hi_edge = np.where(np.isnan(hi_edge), Fv(lo_i), hi_edge)

    # exactness checks (cheap, vectorized)
    v = valid
    assert cond(np.where(v, lo_edge, anchor)).all()
    assert cond(np.where(v, hi_edge, anchor)).all()
    below = np.nextafter(lo_edge, np.float32(-np.inf))
    above = np.nextafter(hi_edge, np.float32(np.inf))
    assert not (v & (below >= xmin) & cond(below)).any()
    assert not (v & (above <= xmax) & cond(above)).any()
    return lo_edge, hi_edge, valid


def _build_regions(traj, eps, xmin, xmax):
    """First-claim partition of [xmin, xmax] into regions of constant k*.
    Returns sorted left edges L (fp32) and per-region kstar (== T: never)."""
    xl, xr, valid = _match_intervals(traj, eps, xmin, xmax)
    down = lambda a: np.nextafter(a, np.float32(-np.inf))  # noqa: E731
    up = lambda a: np.nextafter(a, np.float32(np.inf))     # noqa: E731
    uncovered = [(np.float32(xmin), np.float32(xmax))]
    regions = []
    for k in range(T):
        if not uncovered:
            break
        if not valid[k]:
            continue
        lo_k, hi_k = xl[k], xr[k]
        new_unc = []
        for (a, b) in uncovered:
            if lo_k > b or hi_k < a:
                new_unc.append((a, b))
                continue
            ra, rb = max(lo_k, a), min(hi_k, b)
            regions.append((ra, k))
            if a < ra:
                new_unc.append((a, down(ra)))
            if rb < b:
                new_unc.append((up(rb), b))
        uncovered = new_unc
    for (a, b) in uncovered:
        regions.append((a, T))
    regions.sort(key=lambda r: r[0])
    L = np.array([r[0] for r in regions], np.float32)
    ks = np.array([r[1] for r in regions], np.int64)
    return L, ks


def _region_features(traj, thr, ks):
    """Per-region (tt, energy, p, ent) with the reference's fp32 accumulation
    semantics (sequential fp32 cumsum == per-step fp32 adds)."""
    thr = np.float32(thr)
    t2 = traj * traj                                  # fp32 squares
    Ecum = np.cumsum(t2, dtype=np.float32)            # sequential fp32 adds
    gt = (traj > thr).astype(np.float32)
    Ccum = np.cumsum(gt, dtype=np.float32)            # exact small ints

    fired = ks < T
    j = np.where(fired, ks, T - 1)
    tt = np.where(fired, ks + 1, T).astype(np.float32)
    en = Ecum[j].astype(np.float32)
    cnt = Ccum[j].astype(np.float32)
    p = (cnt / tt).astype(np.float32)

    def xlog2x(v):
        safe = np.where(v > 0, v, np.float32(1.0)).astype(np.float32)
        return np.where(v > 0, v * np.log2(safe, dtype=np.float32),
                        np.float32(0.0)).astype(np.float32)

    ent = -(xlog2x(p) + xlog2x((np.float32(1.0) - p).astype(np.float32)))
    return tt, en, p, ent.astype(np.float32)


def _build_tables(x, ic, thr, eps, W, b):
    """Builds all device-side tables.  Row split: the DVE path
    (scalar_tensor_tensor telescoping) covers n < 4*q_dve; the PE path
    (fp16 hi/lo pair matmuls over rank indicators) covers the rest, for
    all 32 features."""
    traj = _build_traj(ic, thr)
    L, ks = _build_regions(traj, eps, float(x.min()), float(x.max()))
    tt, en, p, ent = _region_features(traj, thr, ks)
    M = L.shape[0]

    # Phi[m, 2f+c] = W[c,4f]*tt + W[c,4f+1]*en + W[c,4f+2]*p + W[c,4f+3]*ent
    W64 = W.astype(np.float64).reshape(2, F, 4)
    feats64 = np.stack([tt, en, p, ent], -1).astype(np.float64)   # [M, 4]
    phi = np.einsum("mj,cfj->mcf", feats64, W64)                  # [M, 2, F]
    phi = phi.transpose(0, 2, 1).reshape(M, 2 * F)                # [M, 64]

    # compensated fp32 deltas: partial fp32 sums track the fp64 table
    dphi = np.empty((M, 2 * F), np.float32)
    running = np.zeros(2 * F, np.float64)
    for m in range(M):
        d = (phi[m] - running).astype(np.float32)
        dphi[m] = d
        running += d.astype(np.float64)

    # pad M to a multiple of 32 free-dim elements; L pad = +inf (never <= x)
    mp = max(32, ((M + 31) // 32) * 32)
    L_pad = np.full(mp, np.float32(np.inf), np.float32)
    L_pad[:M] = L
    dphi_pad = np.zeros((mp, 2 * F), np.float32)
    dphi_pad[:M] = dphi
    per_f = dphi_pad.reshape(mp, F, 2)                 # [m, f, c]

    # ---- DVE-path layouts (partition p = f + 32*r, r = n % 4) ----
    #   lb   [128, mp]      L replicated across partitions
    #   dstt [2, 128, mp]   dstt[c][f+32r, m] = dPhi_{c,f}[m]
    #   s8   [2, 128, 8]    reduction stationary: S_c[f+32r, c+2r] = 1
    #   bias8 [8, 1]        bias8[c+2r] = b[c]
    lb = np.broadcast_to(L_pad, (128, mp)).copy()
    dstt = np.zeros((2, 128, mp), np.float32)
    s8 = np.zeros((2, 128, 8), np.float32)
    for c in range(2):
        for r in range(4):
            for f in range(F):
                dstt[c, f + 32 * r, :] = per_f[:, f, c]
                s8[c, f + 32 * r, c + 2 * r] = 1.0
    bias8 = np.empty((8, 1), np.float32)
    for r in range(4):
        for c in range(2):
            bias8[c + 2 * r, 0] = b[c]

    # ---- PE-path layouts (contraction over m, fp16 hi/lo pair) ----
    #   lpe  [mp, 1]       region edges down the partitions
    #   whi  [mp, 4*F]     stationary: cols 4f.. = (hi_c0, hi_c1, lo_c0, lo_c1)
    lpe = L_pad.reshape(mp, 1).copy()
    hi16 = per_f.reshape(mp, 2 * F).astype(np.float16)
    lo16 = (per_f.reshape(mp, 2 * F).astype(np.float64)
            - hi16.astype(np.float64)).astype(np.float16)
    whi = np.empty((mp, 4 * F), np.float16)
    for f in range(F):
        whi[:, 4 * f:4 * f + 2] = hi16[:, 2 * f:2 * f + 2]
        whi[:, 4 * f + 2:4 * f + 4] = lo16[:, 2 * f:2 * f + 2]
    return lb, dstt, s8, bias8, lpe, whi, mp


# ----------------------------------------------------------------------------
# Device kernel
# ----------------------------------------------------------------------------

NCOL = N_LOC // 4              # 64 element-columns of 128 per core


def _build_device_program(mp, q):
    """q = columns (of 128 elements) on the DVE path; rows n >= 4q go to
    the PE path for all F features."""
    npe = N_LOC - 4 * q         # PE-path rows
    nc = bacc.Bacc("TRN2", target_bir_lowering=False, debug=False,
                   num_devices=NCORES)
    f32 = mybir.dt.float32
    f16 = mybir.dt.float16
    is_le = mybir.AluOpType.is_le
    is_ge = mybir.AluOpType.is_ge
    mult = mybir.AluOpType.mult
    add = mybir.AluOpType.add

    if q:
        xc_d = nc.dram_tensor("xc", [128, q], f32, kind="ExternalInput").ap()
        lb_d = nc.dram_tensor("lb", [128, mp], f32, kind="ExternalInput").ap()
        d0_d = nc.dram_tensor("d0", [128, mp], f32, kind="ExternalInput").ap()
        d1_d = nc.dram_tensor("d1", [128, mp], f32, kind="ExternalInput").ap()
        s8_d = nc.dram_tensor("s8", [2, 128, 8], f32,
                              kind="ExternalInput").ap()
        bias_d = nc.dram_tensor("bias", [8, 1], f32,
                                kind="ExternalInput").ap()
        out_d = nc.dram_tensor("out", [8, q], f32, kind="ExternalOutput").ap()
    if npe:
        epe = F * npe           # elements on the PE path
        xf_d = nc.dram_tensor("xf", [1, epe], f32, kind="ExternalInput").ap()
        lpe_d = nc.dram_tensor("lpe", [mp, 1], f32, kind="ExternalInput").ap()
        whi_d = nc.dram_tensor("whi", [mp, 4 * F], f16,
                               kind="ExternalInput").ap()
        ope_d = nc.dram_tensor("outpe", [4, npe], f32,
                               kind="ExternalOutput").ap()

    n_slice = 2
    with tile.TileContext(nc) as tc, ExitStack() as ctx:
        consts = ctx.enter_context(tc.tile_pool(name="consts", bufs=1))
        scr = ctx.enter_context(tc.tile_pool(name="scr", bufs=1))
        gp = ctx.enter_context(tc.tile_pool(name="g", bufs=1))
        outp = ctx.enter_context(tc.tile_pool(name="outp", bufs=1))
        psum = ctx.enter_context(tc.tile_pool(name="psum", bufs=2,
                                              space="PSUM"))

        # spread input DMAs over the two HWDGE queues (SP + Activation),
        # compute dependencies first
        if npe:
            xf = consts.tile([1, epe], f32, tag="xf")
            nc.scalar.dma_start(xf[:, :], xf_d)
            lpe = consts.tile([mp, 1], f32, tag="lpe")
            nc.scalar.dma_start(lpe[:, :], lpe_d)
            whi = consts.tile([mp, 4 * F], f16, tag="whi")
            nc.scalar.dma_start(whi[:, :], whi_d)
        if q:
            xc = consts.tile([128, q], f32, tag="xc")
            nc.sync.dma_start(xc[:, :], xc_d)
            lb = consts.tile([128, mp], f32, tag="lb")
            nc.sync.dma_start(lb[:, :], lb_d)
            dstt = [consts.tile([128, mp], f32, tag=f"d{c}", name=f"dstt{c}")
                    for c in range(2)]
            nc.sync.dma_start(dstt[0][:, :], d0_d)
            nc.sync.dma_start(dstt[1][:, :], d1_d)
            s8 = [consts.tile([128, 8], f32, tag=f"s8_{c}", name=f"s8t{c}")
                  for c in range(2)]
            for c in range(2):
                nc.scalar.dma_start(s8[c][:, :], s8_d[c])
            bias = consts.tile([8, 1], f32, tag="bias")
            nc.scalar.dma_start(bias[:, :], bias_d)

        # ---- PE path: broadcast x, compare to region edges, fp16 matmuls
        if npe:
            accpe = psum.tile([4, npe], f32, tag="accpe")
            xb = gp.tile([mp, epe], f32, tag="xb")
            u16 = gp.tile([mp, epe], f16, tag="u16")
            f_per = [F // n_slice + (1 if s < F % n_slice else 0)
                     for s in range(n_slice)]
            f0 = 0
            for s in range(n_slice):
                sl = slice(f0 * npe, (f0 + f_per[s]) * npe)
                nc.gpsimd.partition_broadcast(xb[:, sl], xf[:, sl])
                nc.vector.tensor_scalar(u16[:, sl], xb[:, sl], lpe[:, :],
                                        None, is_ge)
                f0 += f_per[s]
            for f in range(F):
                usl = u16[:, f * npe:(f + 1) * npe]
                nc.tensor.matmul(accpe[:, :], whi[:, 4 * f:4 * f + 4], usl,
                                 start=(f == 0), stop=(f == F - 1))
            outpe = outp.tile([4, npe], f32, tag="outpe")
            nc.scalar.mul(outpe[:, :], accpe[:, :], 1.0)
            nc.scalar.dma_start(ope_d, outpe[:, :])

        # ---- DVE path: per-column telescoped rank sums for n < 4q
        if q:
            g = {}
            scratch = {}
            for c in range(2):
                g["v", c] = gp.tile([128, q], f32, tag=f"gv{c}",
                                    name=f"gv{c}")
                scratch["v", c] = scr.tile([128, mp], f32, tag=f"sv{c}",
                                           name=f"sv{c}")
            for c in range(2):
                for col in range(q):
                    xs = xc[:, col:col + 1]
                    nc.vector.scalar_tensor_tensor(
                        scratch["v", c][:, :], lb[:, :], xs, dstt[c][:, :],
                        is_le, mult, accum_out=g["v", c][:, col:col + 1])

            acc = psum.tile([8, q], f32, tag="acc8")
            nc.tensor.matmul(acc[:, :], s8[0][:, :], g["v", 0][:, :],
                             start=True, stop=False)
            nc.tensor.matmul(acc[:, :], s8[1][:, :], g["v", 1][:, :],
                             start=False, stop=True)

            outs = outp.tile([8, q], f32)
            nc.vector.tensor_scalar(outs[:, :], acc[:, :], bias[:, :],
                                    None, add)
            nc.sync.dma_start(out_d, outs[:, :])

    nc.compile()
    return nc


# ----------------------------------------------------------------------------
# Entry point
# ----------------------------------------------------------------------------

def kernel(x, initial_cond, threshold, epsilon, W, b):
    global LAST_RESULTS, LAST_NC
    x = np.ascontiguousarray(np.asarray(x, np.float32))
    W = np.asarray(W, np.float32)
    b = np.asarray(b, np.float32)
    ic = float(np.asarray(initial_cond).reshape(-1)[0])
    thr = float(np.asarray(threshold).reshape(-1)[0])
    eps = float(np.asarray(epsilon).reshape(-1)[0])

    q = int(os.environ.get("Q_DVE", "12"))
    npe = N_LOC - 4 * q
    lb, dstt, s8, bias8, lpe, whi, mp = _build_tables(x, ic, thr, eps, W, b)

    nc = _build_device_program(mp, q)
    LAST_NC = nc

    in_maps = []
    for d in range(NCORES):
        xd = x[d * N_LOC:(d + 1) * N_LOC, :]            # [256, 32]
        im = {}
        if q:
            # xc[f + 32r, col] = x[4*col + r, f]  for n < 4q
            im.update({
                "xc": np.ascontiguousarray(
                    xd[:4 * q].reshape(q, 4, F).transpose(1, 2, 0)
                    .reshape(128, q)),
                "lb": lb, "d0": dstt[0], "d1": dstt[1],
                "s8": s8, "bias": bias8,
            })
        if npe:
            # f-major x for the PE path: xf[0, f*npe + j] = x[4q + j, f]
            im.update({
                "xf": np.ascontiguousarray(xd[4 * q:].T).reshape(1, F * npe),
                "lpe": lpe, "whi": whi,
            })
        in_maps.append(im)

    res = run_bass_kernel_spmd(nc, in_maps, core_ids=list(range(NCORES)))
    LAST_RESULTS = res

    out = np.empty((N, 2), np.float32)
    for d in range(NCORES):
        row0 = d * N_LOC
        if q:
            o8 = res.results[d]["out"]                  # [8, q]: [c+2r, col]
            out[row0:row0 + 4 * q, :] = (
                o8.reshape(4, 2, q).transpose(2, 0, 1).reshape(4 * q, 2))
        if npe:
            o4 = res.results[d]["outpe"]                # [4, npe] hi/lo rows
            out[row0 + 4 * q:row0 + N_LOC, :] = (
                (o4[:2] + o4[2:]).T + b.reshape(1, 2))
    return out



# revision 7
# speedup vs baseline: 1.1849x; 1.1849x over previous
"""ChaosNet (ChaosFEX + linear head) Trainium2 kernel — v2.

Math restructure (unchanged from v1): every per-element feature depends only
on k*(x) = first trajectory index k with |traj[k] - x| < eps.  k*(x) is
piecewise-constant in x (first-claim intervals of the shared trajectory), so

    out[n, c] = b_c + sum_f Phi_{c,f}(k*(x[n,f]))

is, per (c, f), a piecewise-constant function of x with M segments.  With
region left-edges L_0 <= ... <= L_{M-1} and telescoped deltas dPhi:

    Phi(x) = sum_m [x >= L_m] * dPhi[m]

v2 device mapping (per core, 256 rows of x):
  - ONE packed input DMA (v1 used 12; each DMA costs ~625ns of serialized
    HWDGE descriptor generation).
  - The x-broadcast across the mp region-partitions is split across TWO
    producers running in parallel:
      * DMA engines: stride-0 DRAM reads replicate x rows (1.07 ns/elem)
      * gpsimd partition_broadcast (1.43 ns/elem)
  - The rank compare u[m, j] = (x_j >= L_m) is split across TWO engines:
      * DVE tensor_scalar is_ge -> {0, 1} fp16          (0.52 ns/elem)
      * ACT activation Sign(x - L) -> {-1, +1} fp16     (0.83 ns/elem)
    For Sign-blocks the fp16 hi/lo weights are halved and the constant
    0.5*sum_m dPhi is folded into the host-side bias (u = (u' + 1)/2).
  - Tensor engine contracts over m with fp16 hi/lo weight pairs into PSUM.
  - A small DVE scalar_tensor_tensor path keeps rows 0..4q-1 off the
    broadcast entirely (x stays in natural layout there).
"""

import os
import sys
from contextlib import ExitStack

import numpy as np

sys.path.insert(0, "/opt/trn_rl_repo")

import concourse.bass as bass  # noqa: E402
import concourse.tile as tile  # noqa: E402
from concourse import bacc, mybir  # noqa: E402
from concourse.bass_utils import run_bass_kernel_spmd  # noqa: E402

T = 10000
N = 2048
F = 32
NCORES = 8
N_LOC = N // NCORES            # 256 rows per core

np.seterr(all="ignore")

LAST_RESULTS = None            # BassKernelResults of the most recent run
LAST_NC = None                 # compiled Bass program of the most recent run


# ----------------------------------------------------------------------------
# Host-side preprocessing (identical to v1)
# ----------------------------------------------------------------------------

def _build_traj(ic, thr):
    """fp32 skew-tent trajectory, bit-identical to the jax scan."""
    traj = np.empty(T, np.float32)
    z = np.float32(ic)
    thr = np.float32(thr)
    one = np.float32(1.0)
    omt = np.float32(one - thr)
    for k in range(T):
        traj[k] = z
        z = np.float32(z / thr) if z < thr else np.float32((one - z) / omt)
    return traj


def _sortable(i):
    return np.where(i >= 0, i, i ^ np.int32(0x7FFFFFFF))


def _unsortable(k):
    return np.where(k >= 0, k, k ^ np.int32(0x7FFFFFFF))


def _match_intervals(traj, eps, xmin, xmax):
    """Exact fp32 interval [lo_k, hi_k] of {x in [xmin,xmax] :
    |fl32(traj_k - x)| < eps}; valid[k]=False if empty."""
    eps = np.float32(eps)
    xmin = np.float32(xmin)
    xmax = np.float32(xmax)

    def cond(xs):
        return np.abs(traj - xs.astype(np.float32)) < eps

    anchor = np.clip(traj, xmin, xmax)
    valid = cond(anchor)

    I = lambda f: _sortable(f.view(np.int32))             # noqa: E731
    Fv = lambda k: _unsortable(k).view(np.float32)        # noqa: E731

    def bisect(lo_i, hi_i, need, want_smallest_true):
        for _ in range(40):
            gap = np.where(need, hi_i - lo_i, 0)
            if (gap <= 1).all():
                break
            mid = ((lo_i.astype(np.int64) + hi_i) // 2).astype(np.int32)
            cm = cond(Fv(mid))
            if want_smallest_true:
                hi_i = np.where(need & cm, mid, hi_i)
                lo_i = np.where(need & ~cm, mid, lo_i)
            else:
                lo_i = np.where(need & cm, mid, lo_i)
                hi_i = np.where(need & ~cm, mid, hi_i)
        return lo_i, hi_i

    at_min = cond(np.full(T, xmin, np.float32))
    lo_edge = np.where(at_min, xmin, np.float32(np.nan))
    need = valid & np.isnan(lo_edge)
    lo_i = np.broadcast_to(I(xmin.reshape(1)), (T,)).copy()
    hi_i = I(anchor.copy())
    lo_i, hi_i = bisect(lo_i, hi_i, need, True)
    lo_edge = np.where(np.isnan(lo_edge), Fv(hi_i), lo_edge)

    at_max = cond(np.full(T, xmax, np.float32))
    hi_edge = np.where(at_max, xmax, np.float32(np.nan))
    need = valid & np.isnan(hi_edge)
    lo_i = I(anchor.copy())
    hi_i = np.broadcast_to(I(xmax.reshape(1)), (T,)).copy()
    lo_i, hi_i = bisect(lo_i, hi_i, need, False)
    hi_edge = np.where(np.isnan(hi_edge), Fv(lo_i), hi_edge)

    v = valid
    assert cond(np.where(v, lo_edge, anchor)).all()
    assert cond(np.where(v, hi_edge, anchor)).all()
    below = np.nextafter(lo_edge, np.float32(-np.inf))
    above = np.nextafter(hi_edge, np.float32(np.inf))
    assert not (v & (below >= xmin) & cond(below)).any()
    assert not (v & (above <= xmax) & cond(above)).any()
    return lo_edge, hi_edge, valid


def _build_regions(traj, eps, xmin, xmax):
    """First-claim partition of [xmin, xmax] into regions of constant k*."""
    xl, xr, valid = _match_intervals(traj, eps, xmin, xmax)
    down = lambda a: np.nextafter(a, np.float32(-np.inf))  # noqa: E731
    up = lambda a: np.nextafter(a, np.float32(np.inf))     # noqa: E731
    uncovered = [(np.float32(xmin), np.float32(xmax))]
    regions = []
    for k in range(T):
        if not uncovered:
            break
        if not valid[k]:
            continue
        lo_k, hi_k = xl[k], xr[k]
        new_unc = []
        for (a, b) in uncovered:
            if lo_k > b or hi_k < a:
                new_unc.append((a, b))
                continue
            ra, rb = max(lo_k, a), min(hi_k, b)
            regions.append((ra, k))
            if a < ra:
                new_unc.append((a, down(ra)))
            if rb < b:
                new_unc.append((up(rb), b))
        uncovered = new_unc
    for (a, b) in uncovered:
        regions.append((a, T))
    regions.sort(key=lambda r: r[0])
    L = np.array([r[0] for r in regions], np.float32)
    ks = np.array([r[1] for r in regions], np.int64)
    return L, ks


def _region_features(traj, thr, ks):
    """Per-region (tt, energy, p, ent) with the reference's fp32 semantics."""
    thr = np.float32(thr)
    t2 = traj * traj
    Ecum = np.cumsum(t2, dtype=np.float32)
    gt = (traj > thr).astype(np.float32)
    Ccum = np.cumsum(gt, dtype=np.float32)

    fired = ks < T
    j = np.where(fired, ks, T - 1)
    tt = np.where(fired, ks + 1, T).astype(np.float32)
    en = Ecum[j].astype(np.float32)
    cnt = Ccum[j].astype(np.float32)
    p = (cnt / tt).astype(np.float32)

    def xlog2x(v):
        safe = np.where(v > 0, v, np.float32(1.0)).astype(np.float32)
        return np.where(v > 0, v * np.log2(safe, dtype=np.float32),
                        np.float32(0.0)).astype(np.float32)

    ent = -(xlog2x(p) + xlog2x((np.float32(1.0) - p).astype(np.float32)))
    return tt, en, p, ent.astype(np.float32)


# ----------------------------------------------------------------------------
# Plan: element-space chunking of the PE path
# ----------------------------------------------------------------------------

def _make_plan(q, npe):
    """Chunk layout over the PE-path element space [0, 32*npe), f-major.
    Returns dict with DMA-broadcast chunks, Pool-broadcast chunks, compare
    assignment (engine per chunk, f-aligned), and matmul emission order."""
    fd = int(os.environ.get("FD", "20"))          # f-blocks broadcast by DMA
    # broadcast chunks as (f_start, f_end), ramped sizes for pipelining
    dsplit = [int(s) for s in os.environ.get("DSPLIT", "2,6,12,20").split(",")]
    psplit = [int(s) for s in os.environ.get("PSPLIT", "23,27,32").split(",")]
    dchunks = []
    f0 = 0
    for f1 in dsplit:
        f1 = min(f1, fd)
        if f1 > f0:
            dchunks.append((f0, f1))
        f0 = f1
    if f0 < fd:
        dchunks.append((f0, fd))
    pchunks = []
    f0 = fd
    for f1 in psplit:
        f1 = max(min(f1, 32), f0)
        if f1 > f0:
            pchunks.append((f0, f1))
        f0 = f1
    if f0 < 32:
        pchunks.append((f0, 32))

    # compare engine per chunk: "v" (DVE is_ge) or "a" (ACT sign)
    # default: first/smallest DMA chunks -> DVE, large middle ones -> ACT,
    # last Pool chunk -> DVE (ACT window closes earlier).
    n_ch = len(dchunks) + len(pchunks)
    default_asn = "v,v,a,a" + ",a,v,v"[:max(0, 3 * (len(pchunks)) - 2)]
    asn = os.environ.get("CMP_ASN", None)
    if asn is None:
        lst = []
        for i, _ in enumerate(dchunks):
            lst.append("v" if i < 2 else "a")
        for i, _ in enumerate(pchunks):
            lst.append("a" if i == len(pchunks) - 2 else "v")
        asn = ",".join(lst)
    engines = asn.split(",")
    assert len(engines) == n_ch, (asn, n_ch)
    chunks = [(a, b, "dma") for (a, b) in dchunks] + \
             [(a, b, "pool") for (a, b) in pchunks]
    chunks = [(a, b, src, eng) for (a, b, src), eng in zip(chunks, engines)]

    act_f = set()
    for (a, b, src, eng) in chunks:
        if eng == "a":
            act_f.update(range(a, b))
    return {"chunks": chunks, "fd": fd, "act_f": act_f}


# ----------------------------------------------------------------------------
# Device-side tables
# ----------------------------------------------------------------------------

def _build_tables(x, ic, thr, eps, W, b, q, plan):
    traj = _build_traj(ic, thr)
    L, ks = _build_regions(traj, eps, float(x.min()), float(x.max()))
    tt, en, p, ent = _region_features(traj, thr, ks)
    M = L.shape[0]

    W64 = W.astype(np.float64).reshape(2, F, 4)
    feats64 = np.stack([tt, en, p, ent], -1).astype(np.float64)   # [M, 4]
    phi = np.einsum("mj,cfj->mcf", feats64, W64)                  # [M, 2, F]
    phi = phi.transpose(0, 2, 1).reshape(M, 2 * F)                # [M, 64]

    # compensated fp32 deltas: partial fp32 sums track the fp64 table
    dphi = np.empty((M, 2 * F), np.float32)
    running = np.zeros(2 * F, np.float64)
    for m in range(M):
        d = (phi[m] - running).astype(np.float32)
        dphi[m] = d
        running += d.astype(np.float64)

    mp = max(32, ((M + 31) // 32) * 32)
    L_pad = np.full(mp, np.float32(np.inf), np.float32)
    L_pad[:M] = L
    dphi_pad = np.zeros((mp, 2 * F), np.float32)
    dphi_pad[:M] = dphi
    per_f = dphi_pad.reshape(mp, F, 2)                 # [m, f, c]

    act_f = plan["act_f"]

    # ---- STT-path layouts (partition p = f + 32*r, r = n % 4) ----
    dstt = np.zeros((2, 128, mp), np.float32)
    s8 = np.zeros((2, 128, 8), np.float32)
    for c in range(2):
        for r in range(4):
            for f in range(F):
                dstt[c, f + 32 * r, :] = per_f[:, f, c]
                s8[c, f + 32 * r, c + 2 * r] = 1.0
    bias8 = np.zeros((128, 1), np.float32)
    for r in range(4):
        for c in range(2):
            bias8[c + 2 * r, 0] = b[c]

    # ---- PE-path weights: fp16 hi/lo pairs, halved for Sign blocks ----
    whi = np.zeros((mp, 4 * F), np.float16)
    badj = b.astype(np.float64).copy()
    for f in range(F):
        s = 0.5 if f in act_f else 1.0
        base = per_f[:, f, :].astype(np.float64) * s          # [mp, 2]
        hi = base.astype(np.float16)
        lo = (base - hi.astype(np.float64)).astype(np.float16)
        whi[:, 4 * f:4 * f + 2] = hi
        whi[:, 4 * f + 2:4 * f + 4] = lo
        if f in act_f:
            badj += 0.5 * per_f[:, f, :].astype(np.float64).sum(axis=0)

    # ---- packed big tensor [128, C] f32 ----
    # columns: xq(q) | lb(mp) | d0(mp) | d1(mp) | s8(16) | bias8(1) |
    #          lpe(1) | lneg(1) | whi(64 f32-packed fp16)
    cw = q + 3 * mp + 16 + 2 + 1
    C = cw + 2 * F
    big = np.zeros((128, C), np.float32)
    big[:, q:q + mp] = np.broadcast_to(L_pad, (128, mp))
    big[:, q + mp:q + 2 * mp] = dstt[0]
    big[:, q + 2 * mp:q + 3 * mp] = dstt[1]
    c4 = q + 3 * mp
    big[:, c4:c4 + 8] = s8[0]
    big[:, c4 + 8:c4 + 16] = s8[1]
    big[:, c4 + 16:c4 + 17] = bias8
    big[:mp, c4 + 17] = L_pad
    big[:mp, c4 + 18] = -L_pad
    big[:mp, cw:cw + 2 * F] = whi.view(np.float32)
    cols = {"xq": 0, "lb": q, "d0": q + mp, "d1": q + 2 * mp,
            "s80": c4, "s81": c4 + 8, "bias8": c4 + 16, "lpe": c4 + 17,
            "lneg": c4 + 18, "whi": cw, "C": C}
    return big, cols, mp, L_pad, badj


# ----------------------------------------------------------------------------
# Device kernel
# ----------------------------------------------------------------------------

def _build_device_program(mp, q, npe, cols, plan):
    epe = F * npe
    nq2 = npe // 2
    C = cols["C"]
    nc = bacc.Bacc("TRN2", target_bir_lowering=False, debug=False,
                   num_devices=NCORES)
    f32 = mybir.dt.float32
    f16 = mybir.dt.float16
    is_le = mybir.AluOpType.is_le
    is_ge = mybir.AluOpType.is_ge
    mult = mybir.AluOpType.mult
    add = mybir.AluOpType.add
    SIGN = mybir.ActivationFunctionType.Sign

    big_d = nc.dram_tensor("big", [128, C], f32, kind="ExternalInput").ap()
    xf_d = nc.dram_tensor("xf", [1, epe], f32, kind="ExternalInput").ap()
    out_d = nc.dram_tensor("out", [8, q + npe], f32,
                           kind="ExternalOutput").ap()

    chunks = plan["chunks"]
    fd = plan["fd"]

    with tile.TileContext(nc) as tc, ExitStack() as ctx:
        consts = ctx.enter_context(tc.tile_pool(name="consts", bufs=1))
        work = ctx.enter_context(tc.tile_pool(name="work", bufs=1))
        outp = ctx.enter_context(tc.tile_pool(name="outp", bufs=1))
        psum = ctx.enter_context(tc.tile_pool(name="psum", bufs=1,
                                              space="PSUM"))

        big = consts.tile([128, C], f32, tag="big")
        xf = consts.tile([1, epe], f32, tag="xf")
        xb = work.tile([mp, epe], f32, tag="xb")
        u16 = work.tile([mp, epe], f16, tag="u16")

        # table slices inside the packed tile
        xq = big[:, cols["xq"]:cols["xq"] + q]
        lb = big[:, cols["lb"]:cols["lb"] + mp]
        d0 = big[:, cols["d0"]:cols["d0"] + mp]
        d1 = big[:, cols["d1"]:cols["d1"] + mp]
        s80 = big[:, cols["s80"]:cols["s80"] + 8]
        s81 = big[:, cols["s81"]:cols["s81"] + 8]
        bias8 = big[0:8, cols["bias8"]:cols["bias8"] + 1]
        lpe = big[0:mp, cols["lpe"]:cols["lpe"] + 1]
        lneg = big[0:mp, cols["lneg"]:cols["lneg"] + 1]
        wap = big[0:mp, cols["whi"]:cols["whi"] + 2 * F].bitcast(f16)

        # ---- input DMAs: packed tables on SP, f-major x on Act ----
        nc.sync.dma_start(big[:, :], big_d)
        nc.scalar.dma_start(xf[:, :], xf_d)

        # ---- broadcast producers ----
        dq = [nc.sync, nc.scalar]
        di = 0
        for (a, b_, src, eng) in chunks:
            sl = slice(a * npe, b_ * npe)
            n = (b_ - a) * npe
            if src == "dma":
                dq[di % 2].dma_start(
                    xb[:, sl], xf_d[0:1, sl].broadcast_to([mp, n]))
                di += 1
            else:
                nc.gpsimd.partition_broadcast(xb[:, sl], xf[0:1, sl])

        # ---- compares + STT path interleaved on DVE ----
        psA = psum.tile([4, npe], f32, tag="psA")
        psC = psum.tile([8, q], f32, tag="psC")

        gv = [work.tile([128, q], f32, tag=f"gv{c}", name=f"gv{c}")
              for c in range(2)]
        scr = [work.tile([128, mp], f32, tag=f"sv{c}", name=f"sv{c}")
               for c in range(2)]

        stt_jobs = [(c, col) for c in range(2) for col in range(q)]
        n_cmp = len(chunks)
        # interleave: a few STTs between compare chunks
        per_gap = max(1, (len(stt_jobs) + n_cmp - 1) // n_cmp)

        def emit_stt(jobs):
            for c, col in jobs:
                xs = xq[:, col:col + 1]
                nc.vector.scalar_tensor_tensor(
                    scr[c][:, :], lb, xs, (d0 if c == 0 else d1),
                    is_le, mult, accum_out=gv[c][:, col:col + 1])

        si = 0
        mm_started = [False, False]
        mm_count = [0, 0]
        n_mm = [fd, 32 - fd]
        for ci, (a, b_, src, eng) in enumerate(chunks):
            sl = slice(a * npe, b_ * npe)
            if eng == "v":
                emit_stt(stt_jobs[si:si + per_gap])
                si += per_gap
                nc.vector.tensor_scalar(u16[:, sl], xb[:, sl], lpe,
                                        None, is_ge)
            else:
                nc.scalar.activation(u16[:, sl], xb[:, sl], SIGN,
                                     bias=lneg, scale=1.0)
            # matmuls for this chunk's f-blocks (single accumulation group)
            for f in range(a, b_):
                mm_count[0] += 1
                nc.tensor.matmul(psA[:, :], wap[:, 4 * f:4 * f + 4],
                                 u16[:, f * npe:(f + 1) * npe],
                                 start=not mm_started[0],
                                 stop=mm_count[0] == F)
                mm_started[0] = True
        emit_stt(stt_jobs[si:])

        # ---- STT-path reduction: 2 matmuls + bias ----
        outT = outp.tile([8, q + npe], f32, tag="outT")
        nc.tensor.matmul(psC[:, :], s80, gv[0][:, :], start=True, stop=False)
        nc.tensor.matmul(psC[:, :], s81, gv[1][:, :], start=False, stop=True)
        nc.vector.tensor_scalar(outT[:, 0:q], psC[:, :], bias8, None, add)

        # ---- PE-path evict (partition starts must match: rows 0-3) ----
        nc.vector.tensor_copy(outT[0:4, q:q + npe], psA[:, :])

        nc.sync.dma_start(out_d, outT[:, :])

    nc.compile()
    return nc


# ----------------------------------------------------------------------------
# Entry point
# ----------------------------------------------------------------------------

def kernel(x, initial_cond, threshold, epsilon, W, b):
    global LAST_RESULTS, LAST_NC
    x = np.ascontiguousarray(np.asarray(x, np.float32))
    W = np.asarray(W, np.float32)
    b = np.asarray(b, np.float32)
    ic = float(np.asarray(initial_cond).reshape(-1)[0])
    thr = float(np.asarray(threshold).reshape(-1)[0])
    eps = float(np.asarray(epsilon).reshape(-1)[0])

    q = int(os.environ.get("Q_DVE", "8"))
    npe = N_LOC - 4 * q
    plan = _make_plan(q, npe)

    big, cols, mp, L_pad, badj = _build_tables(x, ic, thr, eps, W, b, q, plan)

    # Sign-compare returns 0 on an exact tie x == L; fall back to DVE-only
    # compares (u in {0,1}) if any tie exists in the Sign-assigned blocks.
    if plan["act_f"]:
        act_cols = sorted(plan["act_f"])
        xa = x[:, act_cols] if True else x
        if np.isin(xa, L_pad[np.isfinite(L_pad)]).any():
            plan = {"chunks": [(a, b_, src, "v")
                               for (a, b_, src, _e) in plan["chunks"]],
                    "fd": plan["fd"], "act_f": set()}
            big, cols, mp, L_pad, badj = _build_tables(
                x, ic, thr, eps, W, b, q, plan)

    nc = _build_device_program(mp, q, npe, cols, plan)
    LAST_NC = nc

    in_maps = []
    for d in range(NCORES):
        xd = x[d * N_LOC:(d + 1) * N_LOC, :]            # [256, 32]
        bd = big.copy()
        # xq[f + 32r, col] = x[4*col + r, f]  for rows < 4q
        bd[:, 0:q] = (xd[:4 * q].reshape(q, 4, F).transpose(1, 2, 0)
                      .reshape(128, q))
        im = {"big": bd,
              "xf": np.ascontiguousarray(xd[4 * q:].T).reshape(1, F * npe)}
        in_maps.append(im)

    res = run_bass_kernel_spmd(nc, in_maps, core_ids=list(range(NCORES)))
    LAST_RESULTS = res

    nq2 = npe // 2
    out = np.empty((N, 2), np.float32)
    for d in range(NCORES):
        row0 = d * N_LOC
        o = res.results[d]["out"]                       # [8, q + npe]
        o8 = o[:, 0:q]                                  # [c+2r, col]
        out[row0:row0 + 4 * q, :] = (
            o8.reshape(4, 2, q).transpose(2, 0, 1).reshape(4 * q, 2))
        pe4 = o[0:4, q:]                                # [4, npe]
        out[row0 + 4 * q:row0 + N_LOC, :] = (
            (pe4[:2].astype(np.float64) + pe4[2:].astype(np.float64)).T
            + badj.reshape(1, 2)).astype(np.float32)
    return out


# revision 8
# speedup vs baseline: 1.2030x; 1.0153x over previous
"""ChaosNet (ChaosFEX + linear head) Trainium2 kernel — v2.

Math restructure (unchanged from v1): every per-element feature depends only
on k*(x) = first trajectory index k with |traj[k] - x| < eps.  k*(x) is
piecewise-constant in x (first-claim intervals of the shared trajectory), so

    out[n, c] = b_c + sum_f Phi_{c,f}(k*(x[n,f]))

is, per (c, f), a piecewise-constant function of x with M segments.  With
region left-edges L_0 <= ... <= L_{M-1} and telescoped deltas dPhi:

    Phi(x) = sum_m [x >= L_m] * dPhi[m]

v2 device mapping (per core, 256 rows of x):
  - ONE packed input DMA (v1 used 12; each DMA costs ~625ns of serialized
    HWDGE descriptor generation).
  - The x-broadcast across the mp region-partitions is split across TWO
    producers running in parallel:
      * DMA engines: stride-0 DRAM reads replicate x rows (1.07 ns/elem)
      * gpsimd partition_broadcast (1.43 ns/elem)
  - The rank compare u[m, j] = (x_j >= L_m) is split across TWO engines:
      * DVE tensor_scalar is_ge -> {0, 1} fp16          (0.52 ns/elem)
      * ACT activation Sign(x - L) -> {-1, +1} fp16     (0.83 ns/elem)
    For Sign-blocks the fp16 hi/lo weights are halved and the constant
    0.5*sum_m dPhi is folded into the host-side bias (u = (u' + 1)/2).
  - Tensor engine contracts over m with fp16 hi/lo weight pairs into PSUM.
  - A small DVE scalar_tensor_tensor path keeps rows 0..4q-1 off the
    broadcast entirely (x stays in natural layout there).
"""

import os
import sys
from contextlib import ExitStack

import numpy as np

sys.path.insert(0, "/opt/trn_rl_repo")

import concourse.bass as bass  # noqa: E402
import concourse.tile as tile  # noqa: E402
from concourse import bacc, mybir  # noqa: E402
from concourse.bass_utils import run_bass_kernel_spmd  # noqa: E402

T = 10000
N = 2048
F = 32
NCORES = 8
N_LOC = N // NCORES            # 256 rows per core

np.seterr(all="ignore")

LAST_RESULTS = None            # BassKernelResults of the most recent run
LAST_NC = None                 # compiled Bass program of the most recent run


# ----------------------------------------------------------------------------
# Host-side preprocessing (identical to v1)
# ----------------------------------------------------------------------------

def _build_traj(ic, thr):
    """fp32 skew-tent trajectory, bit-identical to the jax scan."""
    traj = np.empty(T, np.float32)
    z = np.float32(ic)
    thr = np.float32(thr)
    one = np.float32(1.0)
    omt = np.float32(one - thr)
    for k in range(T):
        traj[k] = z
        z = np.float32(z / thr) if z < thr else np.float32((one - z) / omt)
    return traj


def _sortable(i):
    return np.where(i >= 0, i, i ^ np.int32(0x7FFFFFFF))


def _unsortable(k):
    return np.where(k >= 0, k, k ^ np.int32(0x7FFFFFFF))


def _match_intervals(traj, eps, xmin, xmax):
    """Exact fp32 interval [lo_k, hi_k] of {x in [xmin,xmax] :
    |fl32(traj_k - x)| < eps}; valid[k]=False if empty."""
    eps = np.float32(eps)
    xmin = np.float32(xmin)
    xmax = np.float32(xmax)

    def cond(xs):
        return np.abs(traj - xs.astype(np.float32)) < eps

    anchor = np.clip(traj, xmin, xmax)
    valid = cond(anchor)

    I = lambda f: _sortable(f.view(np.int32))             # noqa: E731
    Fv = lambda k: _unsortable(k).view(np.float32)        # noqa: E731

    def bisect(lo_i, hi_i, need, want_smallest_true):
        for _ in range(40):
            gap = np.where(need, hi_i - lo_i, 0)
            if (gap <= 1).all():
                break
            mid = ((lo_i.astype(np.int64) + hi_i) // 2).astype(np.int32)
            cm = cond(Fv(mid))
            if want_smallest_true:
                hi_i = np.where(need & cm, mid, hi_i)
                lo_i = np.where(need & ~cm, mid, lo_i)
            else:
                lo_i = np.where(need & cm, mid, lo_i)
                hi_i = np.where(need & ~cm, mid, hi_i)
        return lo_i, hi_i

    at_min = cond(np.full(T, xmin, np.float32))
    lo_edge = np.where(at_min, xmin, np.float32(np.nan))
    need = valid & np.isnan(lo_edge)
    lo_i = np.broadcast_to(I(xmin.reshape(1)), (T,)).copy()
    hi_i = I(anchor.copy())
    lo_i, hi_i = bisect(lo_i, hi_i, need, True)
    lo_edge = np.where(np.isnan(lo_edge), Fv(hi_i), lo_edge)

    at_max = cond(np.full(T, xmax, np.float32))
    hi_edge = np.where(at_max, xmax, np.float32(np.nan))
    need = valid & np.isnan(hi_edge)
    lo_i = I(anchor.copy())
    hi_i = np.broadcast_to(I(xmax.reshape(1)), (T,)).copy()
    lo_i, hi_i = bisect(lo_i, hi_i, need, False)
    hi_edge = np.where(np.isnan(hi_edge), Fv(lo_i), hi_edge)

    v = valid
    assert cond(np.where(v, lo_edge, anchor)).all()
    assert cond(np.where(v, hi_edge, anchor)).all()
    below = np.nextafter(lo_edge, np.float32(-np.inf))
    above = np.nextafter(hi_edge, np.float32(np.inf))
    assert not (v & (below >= xmin) & cond(below)).any()
    assert not (v & (above <= xmax) & cond(above)).any()
    return lo_edge, hi_edge, valid


def _build_regions(traj, eps, xmin, xmax):
    """First-claim partition of [xmin, xmax] into regions of constant k*."""
    xl, xr, valid = _match_intervals(traj, eps, xmin, xmax)
    down = lambda a: np.nextafter(a, np.float32(-np.inf))  # noqa: E731
    up = lambda a: np.nextafter(a, np.float32(np.inf))     # noqa: E731
    uncovered = [(np.float32(xmin), np.float32(xmax))]
    regions = []
    for k in range(T):
        if not uncovered:
            break
        if not valid[k]:
            continue
        lo_k, hi_k = xl[k], xr[k]
        new_unc = []
        for (a, b) in uncovered:
            if lo_k > b or hi_k < a:
                new_unc.append((a, b))
                continue
            ra, rb = max(lo_k, a), min(hi_k, b)
            regions.append((ra, k))
            if a < ra:
                new_unc.append((a, down(ra)))
            if rb < b:
                new_unc.append((up(rb), b))
        uncovered = new_unc
    for (a, b) in uncovered:
        regions.append((a, T))
    regions.sort(key=lambda r: r[0])
    L = np.array([r[0] for r in regions], np.float32)
    ks = np.array([r[1] for r in regions], np.int64)
    return L, ks


def _region_features(traj, thr, ks):
    """Per-region (tt, energy, p, ent) with the reference's fp32 semantics."""
    thr = np.float32(thr)
    t2 = traj * traj
    Ecum = np.cumsum(t2, dtype=np.float32)
    gt = (traj > thr).astype(np.float32)
    Ccum = np.cumsum(gt, dtype=np.float32)

    fired = ks < T
    j = np.where(fired, ks, T - 1)
    tt = np.where(fired, ks + 1, T).astype(np.float32)
    en = Ecum[j].astype(np.float32)
    cnt = Ccum[j].astype(np.float32)
    p = (cnt / tt).astype(np.float32)

    def xlog2x(v):
        safe = np.where(v > 0, v, np.float32(1.0)).astype(np.float32)
        return np.where(v > 0, v * np.log2(safe, dtype=np.float32),
                        np.float32(0.0)).astype(np.float32)

    ent = -(xlog2x(p) + xlog2x((np.float32(1.0) - p).astype(np.float32)))
    return tt, en, p, ent.astype(np.float32)


# ----------------------------------------------------------------------------
# Plan: element-space chunking of the PE path
# ----------------------------------------------------------------------------

def _make_plan(q, npe):
    """Chunk layout over the PE-path element space [0, 32*npe), f-major.
    Returns dict with DMA-broadcast chunks, Pool-broadcast chunks, compare
    assignment (engine per chunk, f-aligned), and matmul emission order."""
    fd = int(os.environ.get("FD", "26"))          # f-blocks broadcast by DMA
    dsplit = [int(s) for s in os.environ.get("DSPLIT", "3,9,16,21,26").split(",")]
    psplit = [int(s) for s in os.environ.get("PSPLIT", "32").split(",")]
    dchunks = []
    f0 = 0
    for f1 in dsplit:
        f1 = min(f1, fd)
        if f1 > f0:
            dchunks.append((f0, f1))
        f0 = f1
    if f0 < fd:
        dchunks.append((f0, fd))
    pchunks = []
    f0 = fd
    for f1 in psplit:
        f1 = max(min(f1, 32), f0)
        if f1 > f0:
            pchunks.append((f0, f1))
        f0 = f1
    if f0 < 32:
        pchunks.append((f0, 32))

    n_ch = len(dchunks) + len(pchunks)
    asn = os.environ.get("CMP_ASN", "v,a,a,v,v,v")
    engines = asn.split(",")
    assert len(engines) == n_ch, (asn, n_ch)
    chunks = [(a, b, "dma") for (a, b) in dchunks] + \
             [(a, b, "pool") for (a, b) in pchunks]
    chunks = [(a, b, src_, eng) for (a, b, src_), eng in zip(chunks, engines)]
    # emission order = expected arrival order (pool chunk lands mid-stream)
    order = [int(i) for i in os.environ.get("EMIT_ORDER", "0,1,2,5,3,4").split(",")]
    assert sorted(order) == list(range(n_ch)), order
    chunks = [chunks[i] for i in order]

    act_f = set()
    for (a, b, src, eng) in chunks:
        if eng == "a":
            act_f.update(range(a, b))
    return {"chunks": chunks, "fd": fd, "act_f": act_f}


# ----------------------------------------------------------------------------
# Device-side tables
# ----------------------------------------------------------------------------

def _build_tables(x, ic, thr, eps, W, b, q, plan):
    traj = _build_traj(ic, thr)
    L, ks = _build_regions(traj, eps, float(x.min()), float(x.max()))
    tt, en, p, ent = _region_features(traj, thr, ks)
    M = L.shape[0]

    W64 = W.astype(np.float64).reshape(2, F, 4)
    feats64 = np.stack([tt, en, p, ent], -1).astype(np.float64)   # [M, 4]
    phi = np.einsum("mj,cfj->mcf", feats64, W64)                  # [M, 2, F]
    phi = phi.transpose(0, 2, 1).reshape(M, 2 * F)                # [M, 64]

    # compensated fp32 deltas: partial fp32 sums track the fp64 table
    dphi = np.empty((M, 2 * F), np.float32)
    running = np.zeros(2 * F, np.float64)
    for m in range(M):
        d = (phi[m] - running).astype(np.float32)
        dphi[m] = d
        running += d.astype(np.float64)

    mp = max(32, ((M + 31) // 32) * 32)
    L_pad = np.full(mp, np.float32(np.inf), np.float32)
    L_pad[:M] = L
    # compare-edge nudge: [x >= L] == [x > nextafter(L,-inf)] unless some x
    # sits exactly at the nudged value (then keep L and let the caller fall
    # back to DVE-only compares, where ties are handled exactly).
    xvals = np.unique(x)
    L_cmp = L_pad.copy()
    tie_ok = True
    for m in np.nonzero(np.isin(L_pad, xvals))[0]:
        cand = np.nextafter(L_pad[m], np.float32(-np.inf))
        if np.isin(cand, xvals):
            tie_ok = False
        else:
            L_cmp[m] = cand
    dphi_pad = np.zeros((mp, 2 * F), np.float32)
    dphi_pad[:M] = dphi
    per_f = dphi_pad.reshape(mp, F, 2)                 # [m, f, c]

    act_f = plan["act_f"]

    # ---- STT-path layouts (partition p = f + 32*r, r = n % 4) ----
    dstt = np.zeros((2, 128, mp), np.float32)
    s8 = np.zeros((2, 128, 8), np.float32)
    for c in range(2):
        for r in range(4):
            for f in range(F):
                dstt[c, f + 32 * r, :] = per_f[:, f, c]
                s8[c, f + 32 * r, c + 2 * r] = 1.0
    bias8 = np.zeros((128, 1), np.float32)
    for r in range(4):
        for c in range(2):
            bias8[c + 2 * r, 0] = b[c]

    # ---- PE-path weights: fp16 hi/lo pairs, halved for Sign blocks ----
    whi = np.zeros((mp, 4 * F), np.float16)
    badj = b.astype(np.float64).copy()
    for f in range(F):
        s = 0.5 if f in act_f else 1.0
        base = per_f[:, f, :].astype(np.float64) * s          # [mp, 2]
        hi = base.astype(np.float16)
        lo = (base - hi.astype(np.float64)).astype(np.float16)
        whi[:, 4 * f:4 * f + 2] = hi
        whi[:, 4 * f + 2:4 * f + 4] = lo
        if f in act_f:
            badj += 0.5 * per_f[:, f, :].astype(np.float64).sum(axis=0)

    # ---- packed big tensor [128, C] f32 ----
    # columns: xq(q) | lb(mp) | d0(mp) | d1(mp) | s8(16) | bias8(1) |
    #          lpe(1) | lneg(1) | whi(64 f32-packed fp16)
    cw = q + 3 * mp + 16 + 2 + 1
    C = cw + 2 * F
    big = np.zeros((128, C), np.float32)
    big[:, q:q + mp] = np.broadcast_to(L_pad, (128, mp))
    big[:, q + mp:q + 2 * mp] = dstt[0]
    big[:, q + 2 * mp:q + 3 * mp] = dstt[1]
    c4 = q + 3 * mp
    big[:, c4:c4 + 8] = s8[0]
    big[:, c4 + 8:c4 + 16] = s8[1]
    big[:, c4 + 16:c4 + 17] = bias8
    big[:mp, c4 + 17] = L_cmp
    big[:mp, c4 + 18] = -L_cmp
    big[:mp, cw:cw + 2 * F] = whi.view(np.float32)
    cols = {"xq": 0, "lb": q, "d0": q + mp, "d1": q + 2 * mp,
            "s80": c4, "s81": c4 + 8, "bias8": c4 + 16, "lpe": c4 + 17,
            "lneg": c4 + 18, "whi": cw, "C": C}
    return big, cols, mp, L_pad, badj, tie_ok


# ----------------------------------------------------------------------------
# Device kernel
# ----------------------------------------------------------------------------

def _build_device_program(mp, q, npe, cols, plan):
    epe = F * npe
    nq2 = npe // 2
    C = cols["C"]
    nc = bacc.Bacc("TRN2", target_bir_lowering=False, debug=False,
                   num_devices=NCORES)
    f32 = mybir.dt.float32
    f16 = mybir.dt.float16
    is_le = mybir.AluOpType.is_le
    is_ge = mybir.AluOpType.is_ge
    mult = mybir.AluOpType.mult
    add = mybir.AluOpType.add
    SIGN = mybir.ActivationFunctionType.Sign

    big_d = nc.dram_tensor("big", [128, C], f32, kind="ExternalInput").ap()
    xf_d = nc.dram_tensor("xf", [1, epe], f32, kind="ExternalInput").ap()
    out_d = nc.dram_tensor("out", [8, q + npe], f32,
                           kind="ExternalOutput").ap()

    chunks = plan["chunks"]
    fd = plan["fd"]

    with tile.TileContext(nc) as tc, ExitStack() as ctx:
        consts = ctx.enter_context(tc.tile_pool(name="consts", bufs=1))
        work = ctx.enter_context(tc.tile_pool(name="work", bufs=1))
        outp = ctx.enter_context(tc.tile_pool(name="outp", bufs=1))
        psum = ctx.enter_context(tc.tile_pool(name="psum", bufs=1,
                                              space="PSUM"))

        big = consts.tile([128, C], f32, tag="big")
        xf = consts.tile([1, epe], f32, tag="xf")
        xb = work.tile([mp, epe], f32, tag="xb")
        u16 = work.tile([mp, epe], f16, tag="u16")

        # table slices inside the packed tile
        xq = big[:, cols["xq"]:cols["xq"] + q]
        lb = big[:, cols["lb"]:cols["lb"] + mp]
        d0 = big[:, cols["d0"]:cols["d0"] + mp]
        d1 = big[:, cols["d1"]:cols["d1"] + mp]
        s80 = big[:, cols["s80"]:cols["s80"] + 8]
        s81 = big[:, cols["s81"]:cols["s81"] + 8]
        bias8 = big[0:8, cols["bias8"]:cols["bias8"] + 1]
        lpe = big[0:mp, cols["lpe"]:cols["lpe"] + 1]
        lneg = big[0:mp, cols["lneg"]:cols["lneg"] + 1]
        wap = big[0:mp, cols["whi"]:cols["whi"] + 2 * F].bitcast(f16)

        # ---- input DMAs: packed tables on SP, f-major x on Act ----
        nc.sync.dma_start(big[:, :], big_d)
        nc.scalar.dma_start(xf[:, :], xf_d)

        # ---- broadcast producers ----
        dq = [nc.sync, nc.scalar]
        di = 0
        for (a, b_, src, eng) in chunks:
            sl = slice(a * npe, b_ * npe)
            n = (b_ - a) * npe
            if src == "dma":
                dq[di % 2].dma_start(
                    xb[:, sl], xf_d[0:1, sl].broadcast_to([mp, n]))
                di += 1
            else:
                nc.gpsimd.partition_broadcast(xb[:, sl], xf[0:1, sl])

        # ---- compares + STT path interleaved on DVE ----
        psA = psum.tile([4, npe], f32, tag="psA")
        psC = psum.tile([8, q], f32, tag="psC")

        gv = [work.tile([128, q], f32, tag=f"gv{c}", name=f"gv{c}")
              for c in range(2)]
        scr = [work.tile([128, mp], f32, tag=f"sv{c}", name=f"sv{c}")
               for c in range(2)]

        stt_jobs = [(c, col) for c in range(2) for col in range(q)]
        n_v = sum(1 for ch in chunks if ch[3] == "v") or 1
        per_gap = max(1, (len(stt_jobs) + n_v - 1) // n_v)

        def emit_stt(jobs):
            for c, col in jobs:
                xs = xq[:, col:col + 1]
                nc.vector.scalar_tensor_tensor(
                    scr[c][:, :], lb, xs, (d0 if c == 0 else d1),
                    is_le, mult, accum_out=gv[c][:, col:col + 1])

        si = 0
        mm_started = [False, False]
        mm_count = [0, 0]
        n_mm = [fd, 32 - fd]
        for ci, (a, b_, src, eng) in enumerate(chunks):
            sl = slice(a * npe, b_ * npe)
            if eng == "v":
                emit_stt(stt_jobs[si:si + per_gap])
                si += per_gap
                nc.vector.tensor_scalar(u16[:, sl], xb[:, sl], lpe,
                                        None, is_ge)
            else:
                nc.scalar.activation(u16[:, sl], xb[:, sl], SIGN,
                                     bias=lneg, scale=1.0)
            # matmuls for this chunk's f-blocks (single accumulation group)
            for f in range(a, b_):
                mm_count[0] += 1
                nc.tensor.matmul(psA[:, :], wap[:, 4 * f:4 * f + 4],
                                 u16[:, f * npe:(f + 1) * npe],
                                 start=not mm_started[0],
                                 stop=mm_count[0] == F)
                mm_started[0] = True
        emit_stt(stt_jobs[si:])

        # ---- STT-path reduction: 2 matmuls + bias ----
        outT = outp.tile([8, q + npe], f32, tag="outT")
        nc.tensor.matmul(psC[:, :], s80, gv[0][:, :], start=True, stop=False)
        nc.tensor.matmul(psC[:, :], s81, gv[1][:, :], start=False, stop=True)
        nc.vector.tensor_scalar(outT[:, 0:q], psC[:, :], bias8, None, add)

        # ---- PE-path evict (partition starts must match: rows 0-3) ----
        nc.vector.tensor_copy(outT[0:4, q:q + npe], psA[:, :])

        nc.sync.dma_start(out_d, outT[:, :])

    nc.compile()
    return nc


# ----------------------------------------------------------------------------
# Entry point
# ----------------------------------------------------------------------------

def kernel(x, initial_cond, threshold, epsilon, W, b):
    global LAST_RESULTS, LAST_NC
    x = np.ascontiguousarray(np.asarray(x, np.float32))
    W = np.asarray(W, np.float32)
    b = np.asarray(b, np.float32)
    ic = float(np.asarray(initial_cond).reshape(-1)[0])
    thr = float(np.asarray(threshold).reshape(-1)[0])
    eps = float(np.asarray(epsilon).reshape(-1)[0])

    q = int(os.environ.get("Q_DVE", "8"))
    npe = N_LOC - 4 * q
    plan = _make_plan(q, npe)

    big, cols, mp, L_pad, badj, tie_ok = _build_tables(
        x, ic, thr, eps, W, b, q, plan)

    # Sign-compare returns 0 on an exact tie x == L; edges are ulp-nudged in
    # _build_tables, so this fallback only fires if the nudge collided.
    if plan["act_f"] and not tie_ok:
        plan = {"chunks": [(a, b_, src, "v")
                           for (a, b_, src, _e) in plan["chunks"]],
                "fd": plan["fd"], "act_f": set()}
        big, cols, mp, L_pad, badj, tie_ok = _build_tables(
            x, ic, thr, eps, W, b, q, plan)

    nc = _build_device_program(mp, q, npe, cols, plan)
    LAST_NC = nc

    in_maps = []
    for d in range(NCORES):
        xd = x[d * N_LOC:(d + 1) * N_LOC, :]            # [256, 32]
        bd = big.copy()
        # xq[f + 32r, col] = x[4*col + r, f]  for rows < 4q
        bd[:, 0:q] = (xd[:4 * q].reshape(q, 4, F).transpose(1, 2, 0)
                      .reshape(128, q))
        im = {"big": bd,
              "xf": np.ascontiguousarray(xd[4 * q:].T).reshape(1, F * npe)}
        in_maps.append(im)

    res = run_bass_kernel_spmd(nc, in_maps, core_ids=list(range(NCORES)))
    LAST_RESULTS = res

    nq2 = npe // 2
    out = np.empty((N, 2), np.float32)
    for d in range(NCORES):
        row0 = d * N_LOC
        o = res.results[d]["out"]                       # [8, q + npe]
        o8 = o[:, 0:q]                                  # [c+2r, col]
        out[row0:row0 + 4 * q, :] = (
            o8.reshape(4, 2, q).transpose(2, 0, 1).reshape(4 * q, 2))
        pe4 = o[0:4, q:]                                # [4, npe]
        out[row0 + 4 * q:row0 + N_LOC, :] = (
            (pe4[:2].astype(np.float64) + pe4[2:].astype(np.float64)).T
            + badj.reshape(1, 2)).astype(np.float32)
    return out


# revision 9
# speedup vs baseline: 1.2781x; 1.0625x over previous
"""ChaosNet (ChaosFEX + linear head) Trainium2 kernel — v2.

Math restructure (unchanged from v1): every per-element feature depends only
on k*(x) = first trajectory index k with |traj[k] - x| < eps.  k*(x) is
piecewise-constant in x (first-claim intervals of the shared trajectory), so

    out[n, c] = b_c + sum_f Phi_{c,f}(k*(x[n,f]))

is, per (c, f), a piecewise-constant function of x with M segments.  With
region left-edges L_0 <= ... <= L_{M-1} and telescoped deltas dPhi:

    Phi(x) = sum_m [x >= L_m] * dPhi[m]

v2 device mapping (per core, 256 rows of x):
  - ONE packed input DMA (v1 used 12; each DMA costs ~625ns of serialized
    HWDGE descriptor generation).
  - The x-broadcast across the mp region-partitions is split across TWO
    producers running in parallel:
      * DMA engines: stride-0 DRAM reads replicate x rows (1.07 ns/elem)
      * gpsimd partition_broadcast (1.43 ns/elem)
  - The rank compare u[m, j] = (x_j >= L_m) is split across TWO engines:
      * DVE tensor_scalar is_ge -> {0, 1} fp16          (0.52 ns/elem)
      * ACT activation Sign(x - L) -> {-1, +1} fp16     (0.83 ns/elem)
    For Sign-blocks the fp16 hi/lo weights are halved and the constant
    0.5*sum_m dPhi is folded into the host-side bias (u = (u' + 1)/2).
  - Tensor engine contracts over m with fp16 hi/lo weight pairs into PSUM.
  - A small DVE scalar_tensor_tensor path keeps rows 0..4q-1 off the
    broadcast entirely (x stays in natural layout there).
"""

import os
import sys
from contextlib import ExitStack

import numpy as np

sys.path.insert(0, "/opt/trn_rl_repo")

import concourse.bass as bass  # noqa: E402
import concourse.tile as tile  # noqa: E402
from concourse import bacc, mybir  # noqa: E402
from concourse.bass_utils import run_bass_kernel_spmd  # noqa: E402

T = 10000
N = 2048
F = 32
NCORES = 8
N_LOC = N // NCORES            # 256 rows per core

np.seterr(all="ignore")

LAST_RESULTS = None            # BassKernelResults of the most recent run
LAST_NC = None                 # compiled Bass program of the most recent run


# ----------------------------------------------------------------------------
# Host-side preprocessing (identical to v1)
# ----------------------------------------------------------------------------

def _build_traj(ic, thr):
    """fp32 skew-tent trajectory, bit-identical to the jax scan."""
    traj = np.empty(T, np.float32)
    z = np.float32(ic)
    thr = np.float32(thr)
    one = np.float32(1.0)
    omt = np.float32(one - thr)
    for k in range(T):
        traj[k] = z
        z = np.float32(z / thr) if z < thr else np.float32((one - z) / omt)
    return traj


def _sortable(i):
    return np.where(i >= 0, i, i ^ np.int32(0x7FFFFFFF))


def _unsortable(k):
    return np.where(k >= 0, k, k ^ np.int32(0x7FFFFFFF))


def _match_intervals(traj, eps, xmin, xmax):
    """Exact fp32 interval [lo_k, hi_k] of {x in [xmin,xmax] :
    |fl32(traj_k - x)| < eps}; valid[k]=False if empty."""
    eps = np.float32(eps)
    xmin = np.float32(xmin)
    xmax = np.float32(xmax)

    def cond(xs):
        return np.abs(traj - xs.astype(np.float32)) < eps

    anchor = np.clip(traj, xmin, xmax)
    valid = cond(anchor)

    I = lambda f: _sortable(f.view(np.int32))             # noqa: E731
    Fv = lambda k: _unsortable(k).view(np.float32)        # noqa: E731

    def bisect(lo_i, hi_i, need, want_smallest_true):
        for _ in range(40):
            gap = np.where(need, hi_i - lo_i, 0)
            if (gap <= 1).all():
                break
            mid = ((lo_i.astype(np.int64) + hi_i) // 2).astype(np.int32)
            cm = cond(Fv(mid))
            if want_smallest_true:
                hi_i = np.where(need & cm, mid, hi_i)
                lo_i = np.where(need & ~cm, mid, lo_i)
            else:
                lo_i = np.where(need & cm, mid, lo_i)
                hi_i = np.where(need & ~cm, mid, hi_i)
        return lo_i, hi_i

    at_min = cond(np.full(T, xmin, np.float32))
    lo_edge = np.where(at_min, xmin, np.float32(np.nan))
    need = valid & np.isnan(lo_edge)
    lo_i = np.broadcast_to(I(xmin.reshape(1)), (T,)).copy()
    hi_i = I(anchor.copy())
    lo_i, hi_i = bisect(lo_i, hi_i, need, True)
    lo_edge = np.where(np.isnan(lo_edge), Fv(hi_i), lo_edge)

    at_max = cond(np.full(T, xmax, np.float32))
    hi_edge = np.where(at_max, xmax, np.float32(np.nan))
    need = valid & np.isnan(hi_edge)
    lo_i = I(anchor.copy())
    hi_i = np.broadcast_to(I(xmax.reshape(1)), (T,)).copy()
    lo_i, hi_i = bisect(lo_i, hi_i, need, False)
    hi_edge = np.where(np.isnan(hi_edge), Fv(lo_i), hi_edge)

    v = valid
    assert cond(np.where(v, lo_edge, anchor)).all()
    assert cond(np.where(v, hi_edge, anchor)).all()
    below = np.nextafter(lo_edge, np.float32(-np.inf))
    above = np.nextafter(hi_edge, np.float32(np.inf))
    assert not (v & (below >= xmin) & cond(below)).any()
    assert not (v & (above <= xmax) & cond(above)).any()
    return lo_edge, hi_edge, valid


def _build_regions(traj, eps, xmin, xmax):
    """First-claim partition of [xmin, xmax] into regions of constant k*."""
    xl, xr, valid = _match_intervals(traj, eps, xmin, xmax)
    down = lambda a: np.nextafter(a, np.float32(-np.inf))  # noqa: E731
    up = lambda a: np.nextafter(a, np.float32(np.inf))     # noqa: E731
    uncovered = [(np.float32(xmin), np.float32(xmax))]
    regions = []
    for k in range(T):
        if not uncovered:
            break
        if not valid[k]:
            continue
        lo_k, hi_k = xl[k], xr[k]
        new_unc = []
        for (a, b) in uncovered:
            if lo_k > b or hi_k < a:
                new_unc.append((a, b))
                continue
            ra, rb = max(lo_k, a), min(hi_k, b)
            regions.append((ra, k))
            if a < ra:
                new_unc.append((a, down(ra)))
            if rb < b:
                new_unc.append((up(rb), b))
        uncovered = new_unc
    for (a, b) in uncovered:
        regions.append((a, T))
    regions.sort(key=lambda r: r[0])
    L = np.array([r[0] for r in regions], np.float32)
    ks = np.array([r[1] for r in regions], np.int64)
    return L, ks


def _region_features(traj, thr, ks):
    """Per-region (tt, energy, p, ent) with the reference's fp32 semantics."""
    thr = np.float32(thr)
    t2 = traj * traj
    Ecum = np.cumsum(t2, dtype=np.float32)
    gt = (traj > thr).astype(np.float32)
    Ccum = np.cumsum(gt, dtype=np.float32)

    fired = ks < T
    j = np.where(fired, ks, T - 1)
    tt = np.where(fired, ks + 1, T).astype(np.float32)
    en = Ecum[j].astype(np.float32)
    cnt = Ccum[j].astype(np.float32)
    p = (cnt / tt).astype(np.float32)

    def xlog2x(v):
        safe = np.where(v > 0, v, np.float32(1.0)).astype(np.float32)
        return np.where(v > 0, v * np.log2(safe, dtype=np.float32),
                        np.float32(0.0)).astype(np.float32)

    ent = -(xlog2x(p) + xlog2x((np.float32(1.0) - p).astype(np.float32)))
    return tt, en, p, ent.astype(np.float32)


# ----------------------------------------------------------------------------
# Plan: element-space chunking of the PE path
# ----------------------------------------------------------------------------

def _make_plan(q, npe):
    """Chunk layout over the PE-path element space [0, 32*npe), f-major.
    Returns dict with DMA-broadcast chunks, Pool-broadcast chunks, compare
    assignment (engine per chunk, f-aligned), and matmul emission order."""
    fd = int(os.environ.get("FD", "26"))          # f-blocks broadcast by DMA
    dsplit = [int(s) for s in os.environ.get("DSPLIT", "3,9,15,21,26").split(",")]
    psplit = [int(s) for s in os.environ.get("PSPLIT", "32").split(",")]
    dchunks = []
    f0 = 0
    for f1 in dsplit:
        f1 = min(f1, fd)
        if f1 > f0:
            dchunks.append((f0, f1))
        f0 = f1
    if f0 < fd:
        dchunks.append((f0, fd))
    pchunks = []
    f0 = fd
    for f1 in psplit:
        f1 = max(min(f1, 32), f0)
        if f1 > f0:
            pchunks.append((f0, f1))
        f0 = f1
    if f0 < 32:
        pchunks.append((f0, 32))

    n_ch = len(dchunks) + len(pchunks)
    asn = os.environ.get("CMP_ASN", "a,a,a,v,v,v")
    engines = asn.split(",")
    assert len(engines) == n_ch, (asn, n_ch)
    chunks = [(a, b, "dma") for (a, b) in dchunks] + \
             [(a, b, "pool") for (a, b) in pchunks]
    chunks = [(a, b, src_, eng) for (a, b, src_), eng in zip(chunks, engines)]
    # emission order = expected arrival order (pool chunk lands mid-stream)
    order = [int(i) for i in os.environ.get("EMIT_ORDER", "0,1,5,3,4,2").split(",")]
    assert sorted(order) == list(range(n_ch)), order
    chunks = [chunks[i] for i in order]

    act_f = set()
    for (a, b, src, eng) in chunks:
        if eng == "a":
            act_f.update(range(a, b))
    return {"chunks": chunks, "fd": fd, "act_f": act_f}


# ----------------------------------------------------------------------------
# Device-side tables
# ----------------------------------------------------------------------------

def _build_tables(x, ic, thr, eps, W, b, q, plan):
    traj = _build_traj(ic, thr)
    L, ks = _build_regions(traj, eps, float(x.min()), float(x.max()))
    tt, en, p, ent = _region_features(traj, thr, ks)
    M = L.shape[0]

    W64 = W.astype(np.float64).reshape(2, F, 4)
    feats64 = np.stack([tt, en, p, ent], -1).astype(np.float64)   # [M, 4]
    phi = np.einsum("mj,cfj->mcf", feats64, W64)                  # [M, 2, F]
    phi = phi.transpose(0, 2, 1).reshape(M, 2 * F)                # [M, 64]

    # compensated fp32 deltas: partial fp32 sums track the fp64 table
    dphi = np.empty((M, 2 * F), np.float32)
    running = np.zeros(2 * F, np.float64)
    for m in range(M):
        d = (phi[m] - running).astype(np.float32)
        dphi[m] = d
        running += d.astype(np.float64)

    mp = max(32, ((M + 31) // 32) * 32)
    L_pad = np.full(mp, np.float32(np.inf), np.float32)
    L_pad[:M] = L
    # compare-edge nudge: [x >= L] == [x > nextafter(L,-inf)] unless some x
    # sits exactly at the nudged value (then keep L and let the caller fall
    # back to DVE-only compares, where ties are handled exactly).
    xvals = np.unique(x)
    L_cmp = L_pad.copy()
    tie_ok = True
    for m in np.nonzero(np.isin(L_pad, xvals))[0]:
        cand = np.nextafter(L_pad[m], np.float32(-np.inf))
        if np.isin(cand, xvals):
            tie_ok = False
        else:
            L_cmp[m] = cand
    dphi_pad = np.zeros((mp, 2 * F), np.float32)
    dphi_pad[:M] = dphi
    per_f = dphi_pad.reshape(mp, F, 2)                 # [m, f, c]

    act_f = plan["act_f"]

    # ---- STT-path layouts (partition p = f + 32*r, r = n % 4) ----
    dstt = np.zeros((2, 128, mp), np.float32)
    s8 = np.zeros((2, 128, 8), np.float32)
    for c in range(2):
        for r in range(4):
            for f in range(F):
                dstt[c, f + 32 * r, :] = per_f[:, f, c]
                s8[c, f + 32 * r, c + 2 * r] = 1.0
    bias8 = np.zeros((128, 1), np.float32)
    for r in range(4):
        for c in range(2):
            bias8[c + 2 * r, 0] = b[c]

    # ---- PE-path weights: fp16 hi/lo pairs, halved for Sign blocks ----
    whi = np.zeros((mp, 4 * F), np.float16)
    badj = b.astype(np.float64).copy()
    for f in range(F):
        s = 0.5 if f in act_f else 1.0
        base = per_f[:, f, :].astype(np.float64) * s          # [mp, 2]
        hi = base.astype(np.float16)
        lo = (base - hi.astype(np.float64)).astype(np.float16)
        whi[:, 4 * f:4 * f + 2] = hi
        whi[:, 4 * f + 2:4 * f + 4] = lo
        if f in act_f:
            badj += 0.5 * per_f[:, f, :].astype(np.float64).sum(axis=0)

    # ---- packed big tensor [128, C] f32 ----
    # columns: xq(q) | lb(mp) | d0(mp) | d1(mp) | s8(16) | bias8(1) |
    #          lpe(1) | lneg(1) | whi(64 f32-packed fp16)
    cw = q + 3 * mp + 16 + 2 + 1
    C = cw + 2 * F
    big = np.zeros((128, C), np.float32)
    big[:, q:q + mp] = np.broadcast_to(L_pad, (128, mp))
    big[:, q + mp:q + 2 * mp] = dstt[0]
    big[:, q + 2 * mp:q + 3 * mp] = dstt[1]
    c4 = q + 3 * mp
    big[:, c4:c4 + 8] = s8[0]
    big[:, c4 + 8:c4 + 16] = s8[1]
    big[:, c4 + 16:c4 + 17] = bias8
    big[:mp, c4 + 17] = L_cmp
    big[:mp, c4 + 18] = -L_cmp
    big[:mp, cw:cw + 2 * F] = whi.view(np.float32)
    cols = {"xq": 0, "lb": q, "d0": q + mp, "d1": q + 2 * mp,
            "s80": c4, "s81": c4 + 8, "bias8": c4 + 16, "lpe": c4 + 17,
            "lneg": c4 + 18, "whi": cw, "C": C}
    return big, cols, mp, L_pad, badj, tie_ok


# ----------------------------------------------------------------------------
# Device kernel
# ----------------------------------------------------------------------------

def _build_device_program(mp, q, npe, cols, plan):
    epe = F * npe
    nq2 = npe // 2
    C = cols["C"]
    nc = bacc.Bacc("TRN2", target_bir_lowering=False, debug=False,
                   num_devices=NCORES)
    f32 = mybir.dt.float32
    f16 = mybir.dt.float16
    is_le = mybir.AluOpType.is_le
    is_ge = mybir.AluOpType.is_ge
    mult = mybir.AluOpType.mult
    add = mybir.AluOpType.add
    SIGN = mybir.ActivationFunctionType.Sign

    big_d = nc.dram_tensor("big", [128, C], f32, kind="ExternalInput").ap()
    xf_d = nc.dram_tensor("xf", [1, epe], f32, kind="ExternalInput").ap()
    out_d = nc.dram_tensor("out", [8, q + npe], f32,
                           kind="ExternalOutput").ap()

    chunks = plan["chunks"]
    fd = plan["fd"]

    with tile.TileContext(nc) as tc, ExitStack() as ctx:
        consts = ctx.enter_context(tc.tile_pool(name="consts", bufs=1))
        work = ctx.enter_context(tc.tile_pool(name="work", bufs=1))
        outp = ctx.enter_context(tc.tile_pool(name="outp", bufs=1))
        psum = ctx.enter_context(tc.tile_pool(name="psum", bufs=1,
                                              space="PSUM"))

        big = consts.tile([128, C], f32, tag="big")
        xf = consts.tile([1, epe], f32, tag="xf")
        xb = work.tile([mp, epe], f32, tag="xb")
        u16 = work.tile([mp, epe], f16, tag="u16")

        # table slices inside the packed tile
        xq = big[:, cols["xq"]:cols["xq"] + q]
        lb = big[:, cols["lb"]:cols["lb"] + mp]
        d0 = big[:, cols["d0"]:cols["d0"] + mp]
        d1 = big[:, cols["d1"]:cols["d1"] + mp]
        s80 = big[:, cols["s80"]:cols["s80"] + 8]
        s81 = big[:, cols["s81"]:cols["s81"] + 8]
        bias8 = big[0:8, cols["bias8"]:cols["bias8"] + 1]
        lpe = big[0:mp, cols["lpe"]:cols["lpe"] + 1]
        lneg = big[0:mp, cols["lneg"]:cols["lneg"] + 1]
        wap = big[0:mp, cols["whi"]:cols["whi"] + 2 * F].bitcast(f16)

        # ---- input DMAs: packed tables on SP, f-major x on Act ----
        nc.sync.dma_start(big[:, :], big_d)
        nc.scalar.dma_start(xf[:, :], xf_d)

        # ---- broadcast producers ----
        dq = [nc.sync, nc.scalar]
        di = 0
        for (a, b_, src, eng) in chunks:
            sl = slice(a * npe, b_ * npe)
            n = (b_ - a) * npe
            if src == "dma":
                dq[di % 2].dma_start(
                    xb[:, sl], xf_d[0:1, sl].broadcast_to([mp, n]))
                di += 1
            else:
                nc.gpsimd.partition_broadcast(xb[:, sl], xf[0:1, sl])

        # ---- compares + STT path interleaved on DVE ----
        psA = psum.tile([4, npe], f32, tag="psA")
        psC = psum.tile([8, q], f32, tag="psC")

        gv = [work.tile([128, q], f32, tag=f"gv{c}", name=f"gv{c}")
              for c in range(2)]
        scr = [work.tile([128, mp], f32, tag=f"sv{c}", name=f"sv{c}")
               for c in range(2)]

        stt_jobs = [(c, col) for c in range(2) for col in range(q)]
        n_v = sum(1 for ch in chunks if ch[3] == "v") or 1
        per_gap = max(1, (len(stt_jobs) + n_v - 1) // n_v)

        def emit_stt(jobs):
            for c, col in jobs:
                xs = xq[:, col:col + 1]
                nc.vector.scalar_tensor_tensor(
                    scr[c][:, :], lb, xs, (d0 if c == 0 else d1),
                    is_le, mult, accum_out=gv[c][:, col:col + 1])

        si = 0
        mm_started = [False, False]
        mm_count = [0, 0]
        n_mm = [fd, 32 - fd]
        for ci, (a, b_, src, eng) in enumerate(chunks):
            sl = slice(a * npe, b_ * npe)
            if eng == "v":
                emit_stt(stt_jobs[si:si + per_gap])
                si += per_gap
                nc.vector.tensor_scalar(u16[:, sl], xb[:, sl], lpe,
                                        None, is_ge)
            else:
                nc.scalar.activation(u16[:, sl], xb[:, sl], SIGN,
                                     bias=lneg, scale=1.0)
            # matmuls for this chunk's f-blocks (single accumulation group)
            for f in range(a, b_):
                mm_count[0] += 1
                nc.tensor.matmul(psA[:, :], wap[:, 4 * f:4 * f + 4],
                                 u16[:, f * npe:(f + 1) * npe],
                                 start=not mm_started[0],
                                 stop=mm_count[0] == F)
                mm_started[0] = True
        emit_stt(stt_jobs[si:])

        # ---- STT-path reduction: 2 matmuls + bias ----
        outT = outp.tile([8, q + npe], f32, tag="outT")
        nc.tensor.matmul(psC[:, :], s80, gv[0][:, :], start=True, stop=False)
        nc.tensor.matmul(psC[:, :], s81, gv[1][:, :], start=False, stop=True)
        nc.vector.tensor_scalar(outT[:, 0:q], psC[:, :], bias8, None, add)

        # ---- PE-path evict (partition starts must match: rows 0-3) ----
        nc.vector.tensor_copy(outT[0:4, q:q + npe], psA[:, :])

        nc.sync.dma_start(out_d, outT[:, :])

    nc.compile()
    return nc


# ----------------------------------------------------------------------------
# Entry point
# ----------------------------------------------------------------------------

def kernel(x, initial_cond, threshold, epsilon, W, b):
    global LAST_RESULTS, LAST_NC
    x = np.ascontiguousarray(np.asarray(x, np.float32))
    W = np.asarray(W, np.float32)
    b = np.asarray(b, np.float32)
    ic = float(np.asarray(initial_cond).reshape(-1)[0])
    thr = float(np.asarray(threshold).reshape(-1)[0])
    eps = float(np.asarray(epsilon).reshape(-1)[0])

    q = int(os.environ.get("Q_DVE", "8"))
    npe = N_LOC - 4 * q
    plan = _make_plan(q, npe)

    big, cols, mp, L_pad, badj, tie_ok = _build_tables(
        x, ic, thr, eps, W, b, q, plan)

    # Sign-compare returns 0 on an exact tie x == L; edges are ulp-nudged in
    # _build_tables, so this fallback only fires if the nudge collided.
    if plan["act_f"] and not tie_ok:
        plan = {"chunks": [(a, b_, src, "v")
                           for (a, b_, src, _e) in plan["chunks"]],
                "fd": plan["fd"], "act_f": set()}
        big, cols, mp, L_pad, badj, tie_ok = _build_tables(
            x, ic, thr, eps, W, b, q, plan)

    nc = _build_device_program(mp, q, npe, cols, plan)
    LAST_NC = nc

    in_maps = []
    for d in range(NCORES):
        xd = x[d * N_LOC:(d + 1) * N_LOC, :]            # [256, 32]
        bd = big.copy()
        # xq[f + 32r, col] = x[4*col + r, f]  for rows < 4q
        bd[:, 0:q] = (xd[:4 * q].reshape(q, 4, F).transpose(1, 2, 0)
                      .reshape(128, q))
        im = {"big": bd,
              "xf": np.ascontiguousarray(xd[4 * q:].T).reshape(1, F * npe)}
        in_maps.append(im)

    res = run_bass_kernel_spmd(nc, in_maps, core_ids=list(range(NCORES)))
    LAST_RESULTS = res

    nq2 = npe // 2
    out = np.empty((N, 2), np.float32)
    for d in range(NCORES):
        row0 = d * N_LOC
        o = res.results[d]["out"]                       # [8, q + npe]
        o8 = o[:, 0:q]                                  # [c+2r, col]
        out[row0:row0 + 4 * q, :] = (
            o8.reshape(4, 2, q).transpose(2, 0, 1).reshape(4 * q, 2))
        pe4 = o[0:4, q:]                                # [4, npe]
        out[row0 + 4 * q:row0 + N_LOC, :] = (
            (pe4[:2].astype(np.float64) + pe4[2:].astype(np.float64)).T
            + badj.reshape(1, 2)).astype(np.float32)
    return out
